# revision 37
# baseline (speedup 1.0000x reference)
"""Distributed AttentionHead kernel for 8 TRN2 NeuronCores.

Problem: qkv = x @ w.T ; q,k,v = split(qkv); scores[i,j] = k_i.q_j/sqrt(E),
mask keeps j >= i; out = softmax(scores) @ v.   B,N,H,E = 4,2048,1024,1024.

Sharding: core c = 2*b + s handles batch b; it owns the 8 row-tiles
{128*(2*lt+s) : lt in 0..7} (parity interleave => every core's attention
loop has j-extents (16,14,12,...,2) tiles => a single uniform SPMD graph).
Masks that differ between even/odd cores are passed as input *data*,
keeping the graph identical on all cores (collectives-free SPMD).

Algebraic restructure (saves ~2x projection FLOPs vs materializing q/k/v):
  scores = X (Wk^T Wq) X^T = X M X^T    -> T = X_own M, S = T X^T
  out    = P (X Wv^T)   = (P X) Wv^T    -> U = P X, own rows only
M = Wk^T Wq depends only on the weights, so it is folded on the HOST
(f32 matmul, cast bf16) and uploaded as an input -- no on-chip M-phase.

Layout/transfer notes:
- Every DRAM input is staged on the host in its exact SBUF layout
  ([128, free] flat), so each load is a contiguous-row DMA: cheap SWDGE
  descriptor generation and full striping across the 16 rings.
- The T-phase runs hs-OUTER over a dedicated 8-bank PSUM pool, so the
  first matmul needs only the first m/xT column chunks (~0.5 MB), and
  chunk k+1 streams in while chunk k is consumed.
- P^T / U^T come from batched DMA X-bar transposes (~1.3us fixed cost
  per instruction, so one per row-block, not one per tile). For the
  small tail blocks (li>=5) U^T is accumulated directly (U^T = X^T P^T,
  narrow matmuls) and drained on the Vector engine -- keeps the
  end-of-kernel serial chain short; a >3.4us PE idle there would also
  re-throttle the HAM clock gate and double the cost of the last MMs.
"""
import os
import sys

sys.path.insert(0, "/opt/trn_rl_repo")

import numpy as np
import ml_dtypes

import concourse.mybir as mybir
from concourse import bacc
from concourse.tile import TileContext
from concourse.bass_utils import run_bass_kernel_spmd

B, N, H, E = 4, 2048, 1024, 1024
NT = N // 128          # 16 row tiles per batch
LT = 8                 # row tiles owned per core
BF = mybir.dt.bfloat16
F32 = mybir.dt.float32

_CACHE = {}
LAST_RESULT = None


def _build():
    nc = bacc.Bacc("TRN2", target_bir_lowering=False, debug=False, num_devices=8)

    # All inputs pre-staged on the host in SBUF layout: [128, free] flat.
    xT_ext = nc.dram_tensor("xT", [128, 8 * N], BF, kind="ExternalInput")
    xn_ext = nc.dram_tensor("xn", [128, NT * H], BF, kind="ExternalInput")
    m_ext = nc.dram_tensor("m", [128, 8 * H], BF, kind="ExternalInput")
    wvT_ext = nc.dram_tensor("wvT", [128, 8 * E], BF, kind="ExternalInput")
    am_ext = nc.dram_tensor("amask", [128, 256], F32, kind="ExternalInput")
    out_ext = nc.dram_tensor("out", [LT, 128, E], BF, kind="ExternalOutput")

    with TileContext(nc) as tc:
        with (
            tc.tile_pool(name="consts", bufs=1) as consts,
            tc.tile_pool(name="wts", bufs=1) as wts,
            tc.tile_pool(name="bigx", bufs=1) as bigx,
            tc.tile_pool(name="qkv", bufs=1) as qkv,
            tc.tile_pool(name="pbuf", bufs=1) as pbuf,
            tc.tile_pool(name="pts", bufs=2) as ptsp,
            tc.tile_pool(name="ubuf", bufs=3) as ubuf,
            tc.tile_pool(name="utb", bufs=2) as utb,
            tc.tile_pool(name="outb", bufs=2) as outb,
            tc.tile_pool(name="smalls", bufs=3) as smalls,
        ):
            # Warm-up operand memsets FIRST on GpSimd (it starts ~1.5us
            # before Vector, and these must precede the DMA descriptor
            # gens in its FIFO so the PE warmup can begin immediately).
            wu_sb = consts.tile([128, 512], BF)
            wu_lhs = consts.tile([128, 128], BF)
            nc.gpsimd.memset(wu_sb, 0.0)
            nc.gpsimd.memset(wu_lhs, 0.0)

            # Bulk loads next, in consumption order, all SWDGE (gpsimd):
            # m chunk hs and xT own-half chunk hs feed T-phase step hs.
            m_sb = wts.tile([128, 8, H], BF, tag="m", name="m_sb")
            xT_sb = bigx.tile([128, 8, N], BF)
            for hs in range(8):
                nc.gpsimd.dma_start(
                    out=m_sb[:, hs, :], in_=m_ext[:, H * hs:H * hs + H]
                )
                nc.gpsimd.dma_start(
                    out=xT_sb[:, hs, 0:1024],
                    in_=xT_ext[:, N * hs:N * hs + 1024],
                )
            for hs in range(8):
                nc.gpsimd.dma_start(
                    out=xT_sb[:, hs, 1024:2048],
                    in_=xT_ext[:, N * hs + 1024:N * hs + 2048],
                )
            xn_sb = qkv.tile([128, NT, H], BF, tag="xn", name="xn_sb")
            for q in range(4):
                nc.gpsimd.dma_start(
                    out=xn_sb[:, 4 * q:4 * q + 4, :],
                    in_=xn_ext[:, 4 * q * H:4 * (q + 1) * H],
                )
            xn = [xn_sb[:, t, :] for t in range(NT)]
            wvT_sb = bigx.tile([128, 8, E], BF)
            for q in range(2):
                nc.gpsimd.dma_start(
                    out=wvT_sb[:, 4 * q:4 * q + 4, :],
                    in_=wvT_ext[:, 4 * q * E:4 * (q + 1) * E],
                )
            am_sb = consts.tile([128, 256], F32)
            nc.sync.dma_start(out=am_sb, in_=am_ext[:, :])

            TT = [qkv.tile([128, N // 2], BF, tag=f"TT{h}", name=f"TT{h}") for h in range(8)]

            # ------------- T^T = (X_own M)^T = M^T x_own^T (hs-outer) -------------
            with tc.tile_pool(name="tacc", bufs=8, space="PSUM") as taccp:
                wu_ps = taccp.tile([128, 512], F32, tag="tacc", name="wu_ps")
                # 10 warmups: the first input pair is DMA-bound until ~12.3us,
                # and 7 warmups (~2.8us busy) were not enough sustained
                # activity to flip the HAM clock gate -- the first ~5 real T
                # matmuls ran at 1.2GHz. Ten keep the PE busy right up to
                # data arrival and cross the ~3.4us warm threshold.
                for r in range(10):
                    nc.tensor.matmul(wu_ps, wu_lhs, wu_sb, start=True, stop=True)
                # Preload the Exp activation table during warmup: the lazy
                # load (1.3us ACT_TABLE_LOAD) would delay the first real exp.
                wu_act = consts.tile([128, 1], F32)
                nc.scalar.activation(
                    out=wu_act,
                    in_=wu_ps[:, 0:1],
                    func=mybir.ActivationFunctionType.Exp,
                    scale=1.0,
                )

                for half in range(2):
                    i0 = 512 * half
                    pss = [
                        taccp.tile([128, 512], F32, tag="tacc", name=f"ps_t{half}_{ht}")
                        for ht in range(8)
                    ]
                    for hs in range(8):
                        for ht in range(8):
                            nc.tensor.matmul(
                                pss[ht],
                                m_sb[:, hs, 128 * ht:128 * ht + 128],
                                xT_sb[:, hs, i0:i0 + 512],
                                start=hs == 0,
                                stop=hs == 7,
                            )
                    # alternate drain engines: a serial 8-cast DVE chain here
                    # would stall the S-phase's psum-bank reuse by ~4us. In
                    # the second half, drain the middle psums first -- their
                    # banks are the ones the attention S-pool inherits.
                    order = [3, 4, 2, 5, 1, 6, 0, 7] if half == 1 else list(range(8))
                    for k, ht in enumerate(order):
                        if k % 2 == 0:
                            nc.vector.tensor_copy(
                                out=TT[ht][:, i0:i0 + 512], in_=pss[ht]
                            )
                        else:
                            nc.scalar.copy(
                                out=TT[ht][:, i0:i0 + 512], in_=pss[ht]
                            )

                # Bridge the T->S seam: the attention pools' banks can't be
                # written until the T psums drain, so the first TWO S chunks
                # of li=0 are computed HERE, from this pool (rotating into
                # the earliest-drained banks), keeping the PE fed across the
                # seam (~3.6us of matmuls vs ~3us of drain chain).
                s0chunks = []
                for c2 in range(1):
                    sps = taccp.tile([128, 512], F32, tag="tacc", name=f"ps_s0_{c2}")
                    for hs in range(8):
                        rhs0 = xT_sb[:, hs, :].rearrange(
                            "p (two g c) -> p g two c", two=2, c=128
                        )[:, 2 * c2:2 * c2 + 2, :, :]
                        nc.tensor.matmul(
                            sps,
                            TT[hs][:, 0:128],
                            rhs0,
                            start=hs == 0,
                            stop=hs == 7,
                        )
                    s0chunks.append(sps)

            # ---------------- attention ----------------
            # The out-projection of row-block li-1 is emitted between the
            # S-phase and U-phase of block li: its matmuls keep the PE fed
            # while exp(li) runs on the scalar engine.
            with (
                tc.tile_pool(name="acc", bufs=4, space="PSUM") as accp,
                tc.tile_pool(name="sc", bufs=2, space="PSUM") as scp,
                tc.tile_pool(name="up", bufs=2, space="PSUM") as upp,
            ):
                pending_out = None
                pre_chunk = {0: s0chunks[0]}  # li -> precomputed c2=0 psum
                for li in range(LT):
                    nch = 8 - li          # 256-wide score chunks
                    nj = NT - 2 * li      # 128-wide j tiles
                    p = pbuf.tile([128, 256 * nch], BF, tag=f"p{li}", name=f"p{li}")
                    asum = smalls.tile([128, 8], F32, tag="asum", name=f"asum{li}")
                    # 512-wide score chunks (two own/other tile-pairs per psum
                    # group) halve the S accumulation-group count; the rhs dims
                    # are ordered (g, two, c) so p keeps the same
                    # [own g | other g | own g+1 | other g+1] tile order the
                    # U-phase transposes index into.
                    nch2 = (nch + 1) // 2
                    for c2 in range(nch2):
                        g = li + 2 * c2
                        cw = 512 if 2 * c2 + 1 < nch else 256
                        if c2 == 0 and li in pre_chunk:
                            ps = pre_chunk.pop(li)  # precomputed upstream
                        else:
                            ps = scp.tile(
                                [128, cw], F32, tag="sc", name=f"ps_s{li}_{c2}"
                            )
                            for hs in range(8):
                                if cw == 512:
                                    rhs = xT_sb[:, hs, :].rearrange(
                                        "p (two g c) -> p g two c", two=2, c=128
                                    )[:, g:g + 2, :, :]
                                else:
                                    rhs = xT_sb[:, hs, :].rearrange(
                                        "p (two g c) -> p two g c", two=2, c=128
                                    )[:, :, g, :]
                                nc.tensor.matmul(
                                    ps,
                                    TT[hs][:, 128 * li:128 * li + 128],
                                    rhs,
                                    start=hs == 0,
                                    stop=hs == 7,
                                )
                        if c2 == 0:
                            nc.vector.tensor_add(ps[:, 0:256], ps[:, 0:256], am_sb)
                        nc.scalar.activation(
                            out=p[:, 512 * c2:512 * c2 + cw],
                            in_=ps,
                            func=mybir.ActivationFunctionType.Exp,
                            scale=float(1.0 / np.sqrt(E)),
                            accum_out=asum[:, c2:c2 + 1],
                        )
                    # P^T: batched X-bar transposes, laid out [j, u, i] so
                    # ptsT[:, u, :] is tile u of P^T. On Sync (idle; Scalar
                    # still has exp work), split in two for wide blocks so
                    # the U-phase can start on the first half while the last
                    # exp chunks still run.
                    ptsT = ptsp.tile([128, nj, 128], BF, tag="pts", name=f"ptsT{li}")
                    if li < 5:
                        ca = 512 * (nch2 // 2)
                        nc.sync.dma_start_transpose(
                            out=ptsT[:, 0:ca // 128, :], in_=p[:, 0:ca]
                        )
                        nc.sync.dma_start_transpose(
                            out=ptsT[:, ca // 128:nj, :], in_=p[:, ca:128 * nj]
                        )
                    else:
                        # tail blocks: single transpose on Scalar -- it is
                        # idle here (route-B drains run on Vector) and the
                        # issue chains directly behind this block's exp
                        # instead of queueing behind Sync's out-DMAs.
                        nc.scalar.dma_start_transpose(
                            out=ptsT, in_=p[:, 0:128 * nj]
                        )

                    den = smalls.tile([128, 1], F32, tag="den", name=f"den{li}")
                    nc.vector.reduce_sum(den, asum[:, 0:nch2], axis=mybir.AxisListType.X)
                    rden = smalls.tile([128, 1], F32, tag="rden", name=f"rden{li}")
                    nc.vector.reciprocal(rden, den)

                    if li == 0:
                        # li=0 has no deferred out-projection to keep the PE
                        # busy while ptsT(0) transposes; pipeline S(1)'s
                        # first chunk into that slot instead.
                        s1c0 = scp.tile([128, 512], F32, tag="sc", name="ps_s1_0")
                        for hs in range(8):
                            rhs1 = xT_sb[:, hs, :].rearrange(
                                "p (two g c) -> p g two c", two=2, c=128
                            )[:, 1:3, :, :]
                            nc.tensor.matmul(
                                s1c0,
                                TT[hs][:, 128:256],
                                rhs1,
                                start=hs == 0,
                                stop=hs == 7,
                            )
                        pre_chunk[1] = s1c0  # consumed by li=1, c2=0

                    if pending_out is not None:
                        pending_out()
                        pending_out = None

                    # U = P X (f32 accum in PSUM, bf16 out)
                    ut = utb.tile([128, 8, 128], BF, tag="ut", name=f"ut{li}")
                    if li < 5:
                        # wide route: U in two [128,512] psums, drain to SBUF,
                        # one batched U^T DMA transpose; latency hidden by the
                        # next block's S-phase.
                        pv0 = accp.tile([128, 512], F32, tag="acc", name=f"pv0_{li}")
                        pv1 = accp.tile([128, 512], F32, tag="acc", name=f"pv1_{li}")
                        for u in range(nj):
                            jt = (li + u // 2) + (8 if u % 2 else 0)
                            nc.tensor.matmul(
                                pv0, ptsT[:, u, :], xn[jt][:, 0:512],
                                start=u == 0, stop=u == nj - 1,
                            )
                            nc.tensor.matmul(
                                pv1, ptsT[:, u, :], xn[jt][:, 512:1024],
                                start=u == 0, stop=u == nj - 1,
                            )
                        usb = ubuf.tile([128, H], BF, tag="u", name=f"u{li}")
                        nc.scalar.copy(out=usb[:, 0:512], in_=pv0)
                        nc.scalar.copy(out=usb[:, 512:1024], in_=pv1)
                        nc.sync.dma_start_transpose(out=ut, in_=usb)
                    else:
                        # small j-window: accumulate U^T = X^T P^T directly in
                        # narrow psums and drain straight into ut (Vector only:
                        # Scalar must stay free for the next block's exp).
                        for ht in range(8):
                            up = upp.tile([128, 128], F32, tag="up", name=f"up{li}_{ht}")
                            for u in range(nj):
                                jt = (li + u // 2) + (8 if u % 2 else 0)
                                nc.tensor.matmul(
                                    up,
                                    xn[jt][:, 128 * ht:128 * ht + 128],
                                    ptsT[:, u, :],
                                    start=u == 0,
                                    stop=u == nj - 1,
                                )
                            if li == 7 and ht % 2 == 1:
                                # last block: its out-projection waits on ALL
                                # drains with nothing else to run; split them
                                # across Scalar (free here) and Vector.
                                nc.scalar.copy(out=ut[:, ht, :], in_=up)
                            else:
                                nc.vector.tensor_copy(out=ut[:, ht, :], in_=up)

                    # out = U Wv^T, then normalize by the softmax denominator.
                    # cw: psum-group width; the last block uses 256 so the
                    # final drain (vector mul + DMA) after the last matmul is
                    # shorter.
                    def emit_out(li=li, ut=ut, rden=rden, cw=512):
                        ob = outb.tile([128, 1024], BF, tag="ob", name=f"ob{li}")
                        for e0 in range(0, 1024, cw):
                            pool, tg = (accp, "acc") if cw == 512 else (scp, "sc")
                            po = pool.tile([128, cw], F32, tag=tg, name=f"po{li}_{e0}")
                            for hs in range(8):
                                nc.tensor.matmul(
                                    po,
                                    ut[:, hs, :],
                                    wvT_sb[:, hs, e0:e0 + cw],
                                    start=hs == 0,
                                    stop=hs == 7,
                                )
                            nc.vector.tensor_scalar_mul(ob[:, e0:e0 + cw], po, rden)
                            nc.sync.dma_start(
                                out=out_ext[li, :, e0:e0 + cw], in_=ob[:, e0:e0 + cw]
                            )

                    pending_out = emit_out

                # cw=512 for the final block too: at cw=256 the 114ns
                # ut-LDWEIGHTS exceeds the 107ns stream time, so the 32
                # matmuls run LDW-bound at ~131ns (measured); at 512 the
                # weight loads hide completely.
                pending_out(cw=512)

    nc.compile()
    return nc


def _amask(s: int) -> np.ndarray:
    # Additive mask for chunk 0 = [own diagonal tile | partner tile]; the
    # partner tile of slot li is global tile 2li+(1-s): above the diagonal
    # for s=0 (keep), below for s=1 (mask out).
    m = np.zeros((128, 256), dtype=np.float32)
    i = np.arange(128)[:, None]
    j = np.arange(128)[None, :]
    m[:, 0:128] = np.where(j >= i, 0.0, -1e9).astype(np.float32)
    if s == 1:
        m[:, 128:256] = -1e9
    return m


def _perm(s: int) -> np.ndarray:
    own = [2 * u + s for u in range(8)]
    other = [2 * u + 1 - s for u in range(8)]
    return np.array(own + other)


def _to_sbuf_flat(a: np.ndarray) -> np.ndarray:
    """[128*K, F] row-blocked -> [128, K*F] per-partition-contiguous."""
    k = a.shape[0] // 128
    return np.ascontiguousarray(
        a.reshape(k, 128, a.shape[1]).transpose(1, 0, 2).reshape(128, -1)
    )


def kernel(input: np.ndarray, w: np.ndarray) -> np.ndarray:
    global LAST_RESULT
    if "nc" not in _CACHE:
        _CACHE["nc"] = _build()
    nc = _CACHE["nc"]

    bf16 = ml_dtypes.bfloat16
    xb = np.asarray(input, dtype=np.float32).astype(bf16)       # [B, N, H]
    wf = np.asarray(w, dtype=np.float32)                        # [3E, H]
    # Weight-only fold: M = Wk^T Wq in f32 on the host, staged to bf16.
    m = (wf[E:2 * E, :].T @ wf[0:E, :]).astype(bf16)            # [H, H]
    m_flat = _to_sbuf_flat(m)
    wvT_flat = _to_sbuf_flat(
        np.ascontiguousarray(wf[2 * E:3 * E, :].T).astype(bf16)
    )

    in_maps = []
    for c in range(8):
        b, s = divmod(c, 2)
        perm = _perm(s)
        xt3 = xb[b].T.reshape(H, NT, 128)                       # [H, 16, 128]
        xT = xt3[:, perm, :].reshape(H, N)                      # col-tiles permuted
        xn3 = xb[b].reshape(NT, 128, H)
        xn = xn3[perm].reshape(N, H)                            # row-tiles permuted
        in_maps.append(
            {
                "xT": _to_sbuf_flat(xT),
                "xn": _to_sbuf_flat(xn),
                "m": m_flat,
                "wvT": wvT_flat,
                "amask": _amask(s),
            }
        )

    trace = bool(int(os.environ.get("KERNEL_TRACE", "0")))
    res = run_bass_kernel_spmd(nc, in_maps, core_ids=list(range(8)), trace=trace)
    LAST_RESULT = res

    out = np.empty((B, N, E), dtype=np.float32)
    for c in range(8):
        b, s = divmod(c, 2)
        o = np.asarray(res.results[c]["out"], dtype=np.float32)  # [LT, 128, 1024]
        for lt in range(LT):
            r0 = 128 * (2 * lt + s)
            out[b, r0:r0 + 128, :] = o[lt]
    return out
